# revision 20
# baseline (speedup 1.0000x reference)
"""MoE feed-forward block (B=2, T=2048, D=1024, FF=4096, E=8, top-2) on 8 trn2 cores.

Strategy (expert-parallel, matching the sharding hint):
  - Router (x @ Wr.T, top-2, softmax) computed on host in fp64: it is tiny
    (4096x1024x8) and its output is *indices* + weights, i.e. the dispatch.
  - Dispatch: tokens are gathered per expert on host (the all-to-all), padded
    to a common capacity C, and each of the 8 cores runs the FFN of one
    expert over its routed tokens.
  - Device kernel per core: y = gelu(x @ W1) @ W2 in fp16 (fp32 PSUM
    accumulate), over [C, 1024] tokens.
  - Combine: host does out[idx_e] += w_e * y_e (fp32), the weighted
    scatter-add, then reshapes to [B, T, D].

Dataflow on device keeps activations in [feature, token] layout so both GEMMs
use natural-layout weight tiles as the stationary operand:
  GEMM1: Ht[f*128:(f+1)*128, :] = (W1[:, fcols].T @ xT)  via
         matmul(lhsT=W1[dchunk, fcols], rhs=xT[dchunk, ctile])
  gelu:  ACT reads PSUM, writes SBUF fp16.
  GEMM2: Y[ctile, dcols] = sum_f Ht[fchunk, ctile].T @ W2[fchunk, dcols]
W2 (fp16, 8.4 MB) stays resident in SBUF; W1 streams per 128-wide column
block; x and Ht are SBUF-resident.
"""

import sys

sys.path.insert(0, "/opt/trn_rl_repo")

import math
from contextlib import ExitStack

import numpy as np

import concourse.bass as bass
import concourse.tile as tile
from concourse import bacc, mybir
from concourse.bass_utils import run_bass_kernel_spmd

B, T, D, FF, E, TOPK = 2, 2048, 1024, 4096, 8, 2
N_CORES = 8
DC = D // 128  # 8 d-chunks
FC = FF // 128  # 32 ff-chunks

_cache: dict[int, object] = {}


def _c_chunks(C: int) -> list[tuple[int, int]]:
    """Split C into <=512-sized chunks (PSUM bank limit), roughly equal."""
    n = max(1, math.ceil(C / 512))
    base = C // n
    rem = C - base * n
    sizes = [base + (1 if i < rem else 0) for i in range(n)]
    out, off = [], 0
    for s in sizes:
        out.append((off, s))
        off += s
    return out


def _build(C: int, reps: int = 1):
    f16 = mybir.dt.float16
    f32 = mybir.dt.float32
    nc = bacc.Bacc("TRN2", target_bir_lowering=False, debug=False)
    xt = nc.dram_tensor("xt", [D, C], f16, kind="ExternalInput").ap()
    # w1t[f, p, d*128+c] = W1[d*128+p, f*128+c]
    w1t = nc.dram_tensor("w1t", [FC, 128, D], f16, kind="ExternalInput").ap()
    # w2t[f, p, :] = W2[f*128+p, :]
    w2t = nc.dram_tensor("w2t", [FC, 128, D], f16, kind="ExternalInput").ap()
    y = nc.dram_tensor("y", [C, D], f32, kind="ExternalOutput").ap()

    chunks = _c_chunks(C)
    n_cc = len(chunks)
    ps1_bufs = max(1, min(2, (8 - 2) // n_cc))

    with tile.TileContext(nc) as tc:
        for _rep in range(reps):
            _emit(nc, tc, xt, w1t, w2t, y, C, chunks, ps1_bufs, _rep)
    nc.compile()
    return nc


def _emit(nc, tc, xt, w1t, w2t, y, C, chunks, ps1_bufs, rep):
    f16 = mybir.dt.float16
    f32 = mybir.dt.float32
    if True:  # keep original indentation of the pool block
        with ExitStack() as ctx:
            xpool = ctx.enter_context(tc.tile_pool(name="x", bufs=1))
            hpool = ctx.enter_context(tc.tile_pool(name="h", bufs=1))
            w2pool = ctx.enter_context(tc.tile_pool(name="w2", bufs=1))
            w1pool = ctx.enter_context(tc.tile_pool(name="w1", bufs=3))
            ps1pool = ctx.enter_context(tc.tile_pool(name="ps1", bufs=ps1_bufs, space="PSUM"))
            ps2pool = ctx.enter_context(tc.tile_pool(name="ps2", bufs=2, space="PSUM"))
            ypool = ctx.enter_context(tc.tile_pool(name="yp", bufs=3))

            # first GEMM1 weight block goes out ahead of x so PE can start
            # as soon as the first x tile lands.
            w1sb0 = w1pool.tile([128, D], f16, tag="w1sb", name=f"w1sb0_r{rep}")
            nc.sync.dma_start(w1sb0[:], w1t[0])
            xsb = [xpool.tile([128, C], f16, name=f"xsb{d}_r{rep}") for d in range(DC)]
            for d in range(DC):
                nc.sync.dma_start(xsb[d][:], xt[d * 128 : (d + 1) * 128, :])
            w2sb = [w2pool.tile([128, D], f16, name=f"w2sb{f}_r{rep}") for f in range(FC)]
            ht = [hpool.tile([128, C], f16, name=f"ht{f}_r{rep}") for f in range(FC)]

            # GEMM1 + gelu: Ht[f] = gelu(W1[:, fcols].T @ xT). The W2 loads
            # are issued inside this loop (after each f's matmuls) so they
            # stream in behind the W1 tiles instead of delaying PE start.
            for f in range(FC):
                if f == 0:
                    w1sb = w1sb0
                else:
                    w1sb = w1pool.tile([128, D], f16, tag="w1sb", name=f"w1sb{f}_r{rep}")
                    nc.sync.dma_start(w1sb[:], w1t[f])
                pss = [
                    ps1pool.tile([128, clen], f32, tag=f"ps1_{cn}", name=f"ps1_{f}_{cn}_r{rep}")
                    for cn, (coff, clen) in enumerate(chunks)
                ]
                # d outer / c-chunk inner: the first matmul only needs xsb[0]
                # and w1sb rather than all of x. The psum groups accumulate
                # concurrently in separate banks.
                for d in range(DC):
                    for cn, (coff, clen) in enumerate(chunks):
                        nc.tensor.matmul(
                            pss[cn][:],
                            w1sb[:, d * 128 : (d + 1) * 128],
                            xsb[d][:, coff : coff + clen],
                            start=(d == 0),
                            stop=(d == DC - 1),
                        )
                for cn, (coff, clen) in enumerate(chunks):
                    nc.scalar.activation(
                        ht[f][:, coff : coff + clen], pss[cn][:], mybir.ActivationFunctionType.Gelu
                    )
                # delay W2 loads behind the first 8 W1 blocks so the early W1
                # prefetches are never queued behind W2 traffic
                if f >= 8:
                    nc.sync.dma_start(w2sb[f - 8][:], w2t[f - 8])
            for f in range(FC - 8, FC):
                nc.sync.dma_start(w2sb[f][:], w2t[f])

            # GEMM2: Y[ci_tile, dh*512:(dh+1)*512]
            n_ci = (C + 127) // 128
            for ci in range(n_ci):
                coff = ci * 128
                clen = min(128, C - coff)
                for dh in range(2):
                    ps = ps2pool.tile([clen, 512], f32, tag="ps2", name=f"ps2_{ci}_{dh}_r{rep}")
                    for f in range(FC):
                        nc.tensor.matmul(
                            ps[:],
                            ht[f][:, coff : coff + clen],
                            w2sb[f][:, dh * 512 : (dh + 1) * 512],
                            start=(f == 0),
                            stop=(f == FC - 1),
                        )
                    ysb = ypool.tile([clen, 512], f32, tag="ysb", name=f"ysb_{ci}_{dh}_r{rep}")
                    nc.vector.tensor_copy(ysb[:], ps[:])
                    nc.sync.dma_start(
                        y[coff : coff + clen, dh * 512 : (dh + 1) * 512], ysb[:]
                    )


def _route(xf: np.ndarray, Wr: np.ndarray):
    """Host router: top-2 + softmax, fp64 logits for stable decisions."""
    logits = xf.astype(np.float64) @ Wr.astype(np.float64).T  # [N, E]
    top2 = np.argsort(-logits, axis=1, kind="stable")[:, :TOPK]  # [N, 2] desc
    lv = np.take_along_axis(logits, top2, axis=1).astype(np.float32)
    m = lv.max(axis=1, keepdims=True)
    ex = np.exp(lv - m)
    w = (ex / ex.sum(axis=1, keepdims=True)).astype(np.float32)  # [N, 2]
    return top2, w


# SBUF fits x/Ht/W2-resident up to roughly C~1400 tokens per expert; beyond
# that (a >9-sigma routing skew for randn inputs) dispatch in multiple passes.
C_SBUF_MAX = 1400


def _run_pass(xf, W1, W2, idx, wts, out, trace):
    """One SPMD dispatch over the given per-expert token lists."""
    cmax = max((len(t) for t in idx), default=0)
    C = max(256, ((cmax + 1) // 2) * 2)  # even, no 128-padding

    if C not in _cache:
        _cache[C] = _build(C)
    nc = _cache[C]

    in_maps = []
    for e in range(E):
        xt_e = np.zeros((D, C), dtype=np.float16)
        xt_e[:, : len(idx[e])] = xf[idx[e]].T
        w1t_e = (
            np.asarray(W1[e], dtype=np.float16)
            .reshape(DC, 128, FC, 128)
            .transpose(2, 1, 0, 3)
            .reshape(FC, 128, D)
        )
        w1t_e = np.ascontiguousarray(w1t_e)
        w2t_e = np.ascontiguousarray(np.asarray(W2[e], dtype=np.float16).reshape(FC, 128, D))
        in_maps.append({"xt": xt_e, "w1t": w1t_e, "w2t": w2t_e})

    res = run_bass_kernel_spmd(nc, in_maps, list(range(N_CORES)), trace=trace)

    for e in range(E):
        ye = res.results[e]["y"][: len(idx[e])]  # [C_e, D] fp32
        out[idx[e]] += wts[e][:, None] * ye
    return res


def _run(x, Wr, W1, W2, trace=False):
    xf = np.asarray(x, dtype=np.float32).reshape(-1, D)
    N = xf.shape[0]
    top2, tw = _route(xf, np.asarray(Wr, dtype=np.float32))

    idx, wts = [], []
    for e in range(E):
        mask = top2 == e  # [N, 2]
        tok = np.nonzero(mask.any(axis=1))[0]
        # weight for token t is tw[t, k] where top2[t, k] == e
        k = np.argmax(mask[tok], axis=1)
        we = tw[tok, k]
        idx.append(tok)
        wts.append(we.astype(np.float32))

    cmax = max(len(t) for t in idx)
    n_pass = max(1, math.ceil(cmax / C_SBUF_MAX))

    out = np.zeros((N, D), dtype=np.float32)
    res = None
    for p in range(n_pass):
        idx_p = [t[p * len(t) // n_pass : (p + 1) * len(t) // n_pass] for t in idx]
        wts_p = [w[p * len(w) // n_pass : (p + 1) * len(w) // n_pass] for w in wts]
        res = _run_pass(xf, W1, W2, idx_p, wts_p, out, trace)
    return out.reshape(B, T, D), res


def kernel(x, Wr, W1, W2):
    out, _ = _run(x, Wr, W1, W2, trace=False)
    return out


# revision 28
# speedup vs baseline: 1.0010x; 1.0010x over previous
"""MoE feed-forward block (B=2, T=2048, D=1024, FF=4096, E=8, top-2) on 8 trn2 cores.

Strategy (expert-parallel, matching the sharding hint):
  - Router (x @ Wr.T, top-2, softmax) computed on host in fp64: it is tiny
    (4096x1024x8) and its output is *indices* + weights, i.e. the dispatch.
  - Dispatch: tokens are gathered per expert on host (the all-to-all), padded
    to a common capacity C, and each of the 8 cores runs the FFN of one
    expert over its routed tokens.
  - Device kernel per core: y = gelu(x @ W1) @ W2 in fp16 (fp32 PSUM
    accumulate), over [C, 1024] tokens.
  - Combine: host does out[idx_e] += w_e * y_e (fp32), the weighted
    scatter-add, then reshapes to [B, T, D].

Dataflow on device keeps activations in [feature, token] layout so both GEMMs
use natural-layout weight tiles as the stationary operand:
  GEMM1: Ht[f*128:(f+1)*128, :] = (W1[:, fcols].T @ xT)  via
         matmul(lhsT=W1[dchunk, fcols], rhs=xT[dchunk, ctile])
  gelu:  ACT reads PSUM, writes SBUF fp16.
  GEMM2: Y[ctile, dcols] = sum_f Ht[fchunk, ctile].T @ W2[fchunk, dcols]
W2 (fp16, 8.4 MB) stays resident in SBUF; W1 streams per 128-wide column
block; x and Ht are SBUF-resident.
"""

import sys

sys.path.insert(0, "/opt/trn_rl_repo")

import math
from contextlib import ExitStack

import numpy as np

import concourse.bass as bass
import concourse.tile as tile
from concourse import bacc, mybir
from concourse.bass_utils import run_bass_kernel_spmd

B, T, D, FF, E, TOPK = 2, 2048, 1024, 4096, 8, 2
N_CORES = 8
DC = D // 128  # 8 d-chunks
FC = FF // 128  # 32 ff-chunks

_cache: dict[int, object] = {}


def _c_chunks(C: int) -> list[tuple[int, int]]:
    """Split C into <=512-sized chunks (PSUM bank limit), roughly equal."""
    n = max(1, math.ceil(C / 512))
    base = C // n
    rem = C - base * n
    sizes = [base + (1 if i < rem else 0) for i in range(n)]
    out, off = [], 0
    for s in sizes:
        out.append((off, s))
        off += s
    return out


def _build(C: int, reps: int = 1):
    f16 = mybir.dt.float16
    f32 = mybir.dt.float32
    nc = bacc.Bacc("TRN2", target_bir_lowering=False, debug=False)
    xt = nc.dram_tensor("xt", [D, C], f16, kind="ExternalInput").ap()
    # w1t[f, p, d*128+c] = W1[d*128+p, f*128+c]
    w1t = nc.dram_tensor("w1t", [FC, 128, D], f16, kind="ExternalInput").ap()
    # w2t[f, p, :] = W2[f*128+p, :]
    w2t = nc.dram_tensor("w2t", [FC, 128, D], f16, kind="ExternalInput").ap()
    y = nc.dram_tensor("y", [C, D], f32, kind="ExternalOutput").ap()

    chunks = _c_chunks(C)
    n_cc = len(chunks)
    ps1_bufs = max(1, min(2, (8 - 2) // n_cc))

    with tile.TileContext(nc) as tc:
        for _rep in range(reps):
            _emit(nc, tc, xt, w1t, w2t, y, C, chunks, ps1_bufs, _rep)
    nc.compile()
    return nc


def _emit(nc, tc, xt, w1t, w2t, y, C, chunks, ps1_bufs, rep):
    f16 = mybir.dt.float16
    f32 = mybir.dt.float32
    if True:  # keep original indentation of the pool block
        with ExitStack() as ctx:
            xpool = ctx.enter_context(tc.tile_pool(name="x", bufs=1))
            hpool = ctx.enter_context(tc.tile_pool(name="h", bufs=1))
            w2pool = ctx.enter_context(tc.tile_pool(name="w2", bufs=1))
            w1pool = ctx.enter_context(tc.tile_pool(name="w1", bufs=3))
            ps1pool = ctx.enter_context(tc.tile_pool(name="ps1", bufs=ps1_bufs, space="PSUM"))
            ps2pool = ctx.enter_context(tc.tile_pool(name="ps2", bufs=2, space="PSUM"))
            ypool = ctx.enter_context(tc.tile_pool(name="yp", bufs=3))

            # first GEMM1 weight block goes out ahead of x so PE can start
            # as soon as the first x tile lands.
            w1sb0 = w1pool.tile([128, D], f16, tag="w1sb", name=f"w1sb0_r{rep}")
            nc.sync.dma_start(w1sb0[:], w1t[0])
            xsb = [xpool.tile([128, C], f16, name=f"xsb{d}_r{rep}") for d in range(DC)]
            for d in range(DC):
                nc.sync.dma_start(xsb[d][:], xt[d * 128 : (d + 1) * 128, :])
            w2sb = [w2pool.tile([128, D], f16, name=f"w2sb{f}_r{rep}") for f in range(FC)]
            ht = [hpool.tile([128, C], f16, name=f"ht{f}_r{rep}") for f in range(FC)]

            # GEMM1 + gelu: Ht[f] = gelu(W1[:, fcols].T @ xT). The W2 loads
            # are issued inside this loop (after each f's matmuls) so they
            # stream in behind the W1 tiles instead of delaying PE start.
            for f in range(FC):
                if f == 0:
                    w1sb = w1sb0
                else:
                    w1sb = w1pool.tile([128, D], f16, tag="w1sb", name=f"w1sb{f}_r{rep}")
                    nc.sync.dma_start(w1sb[:], w1t[f])
                pss = [
                    ps1pool.tile([128, clen], f32, tag=f"ps1_{cn}", name=f"ps1_{f}_{cn}_r{rep}")
                    for cn, (coff, clen) in enumerate(chunks)
                ]
                # d outer / c-chunk inner: the first matmul only needs xsb[0]
                # and w1sb rather than all of x; the psum groups accumulate
                # concurrently in separate banks
                for d in range(DC):
                    for cn, (coff, clen) in enumerate(chunks):
                        nc.tensor.matmul(
                            pss[cn][:],
                            w1sb[:, d * 128 : (d + 1) * 128],
                            xsb[d][:, coff : coff + clen],
                            start=(d == 0),
                            stop=(d == DC - 1),
                        )
                for cn, (coff, clen) in enumerate(chunks):
                    nc.scalar.activation(
                        ht[f][:, coff : coff + clen], pss[cn][:], mybir.ActivationFunctionType.Gelu
                    )
                # delay W2 loads behind the first 8 W1 blocks so the early W1
                # prefetches are never queued behind W2 traffic
                if f >= 8:
                    nc.sync.dma_start(w2sb[f - 8][:], w2t[f - 8])
            for f in range(FC - 8, FC):
                nc.sync.dma_start(w2sb[f][:], w2t[f])

            # GEMM2: Y[ci_tile, dcols]. The last group is split into two
            # 256-wide halves so its copy+DMA drain overlaps the final matmuls
            # instead of sitting fully exposed at the kernel tail.
            n_ci = (C + 127) // 128
            for ci in range(n_ci):
                coff = ci * 128
                clen = min(128, C - coff)
                dcols = [(0, 512), (512, 512)]
                if ci == n_ci - 1:
                    dcols = [(0, 512), (512, 256), (768, 256)]
                for dh, (doff, dlen) in enumerate(dcols):
                    ps = ps2pool.tile([clen, dlen], f32, tag="ps2", name=f"ps2_{ci}_{dh}_r{rep}")
                    for f in range(FC):
                        nc.tensor.matmul(
                            ps[:],
                            ht[f][:, coff : coff + clen],
                            w2sb[f][:, doff : doff + dlen],
                            start=(f == 0),
                            stop=(f == FC - 1),
                        )
                    ysb = ypool.tile([clen, dlen], f32, tag="ysb", name=f"ysb_{ci}_{dh}_r{rep}")
                    nc.vector.tensor_copy(ysb[:], ps[:])
                    nc.sync.dma_start(
                        y[coff : coff + clen, doff : doff + dlen], ysb[:]
                    )


def _route(xf: np.ndarray, Wr: np.ndarray):
    """Host router: top-2 + softmax, fp64 logits for stable decisions."""
    logits = xf.astype(np.float64) @ Wr.astype(np.float64).T  # [N, E]
    top2 = np.argsort(-logits, axis=1, kind="stable")[:, :TOPK]  # [N, 2] desc
    lv = np.take_along_axis(logits, top2, axis=1).astype(np.float32)
    m = lv.max(axis=1, keepdims=True)
    ex = np.exp(lv - m)
    w = (ex / ex.sum(axis=1, keepdims=True)).astype(np.float32)  # [N, 2]
    return top2, w


# SBUF fits x/Ht/W2-resident up to roughly C~1400 tokens per expert; beyond
# that (a >9-sigma routing skew for randn inputs) dispatch in multiple passes.
C_SBUF_MAX = 1400


def _run_pass(xf, W1, W2, idx, wts, out, trace):
    """One SPMD dispatch over the given per-expert token lists."""
    cmax = max((len(t) for t in idx), default=0)
    C = max(256, ((cmax + 1) // 2) * 2)  # even, no 128-padding

    if C not in _cache:
        _cache[C] = _build(C)
    nc = _cache[C]

    in_maps = []
    for e in range(E):
        xt_e = np.zeros((D, C), dtype=np.float16)
        xt_e[:, : len(idx[e])] = xf[idx[e]].T
        w1t_e = (
            np.asarray(W1[e], dtype=np.float16)
            .reshape(DC, 128, FC, 128)
            .transpose(2, 1, 0, 3)
            .reshape(FC, 128, D)
        )
        w1t_e = np.ascontiguousarray(w1t_e)
        w2t_e = np.ascontiguousarray(np.asarray(W2[e], dtype=np.float16).reshape(FC, 128, D))
        in_maps.append({"xt": xt_e, "w1t": w1t_e, "w2t": w2t_e})

    res = run_bass_kernel_spmd(nc, in_maps, list(range(N_CORES)), trace=trace)

    for e in range(E):
        ye = res.results[e]["y"][: len(idx[e])]  # [C_e, D] fp32
        out[idx[e]] += wts[e][:, None] * ye
    return res


def _run(x, Wr, W1, W2, trace=False):
    xf = np.asarray(x, dtype=np.float32).reshape(-1, D)
    N = xf.shape[0]
    top2, tw = _route(xf, np.asarray(Wr, dtype=np.float32))

    idx, wts = [], []
    for e in range(E):
        mask = top2 == e  # [N, 2]
        tok = np.nonzero(mask.any(axis=1))[0]
        # weight for token t is tw[t, k] where top2[t, k] == e
        k = np.argmax(mask[tok], axis=1)
        we = tw[tok, k]
        idx.append(tok)
        wts.append(we.astype(np.float32))

    cmax = max(len(t) for t in idx)
    n_pass = max(1, math.ceil(cmax / C_SBUF_MAX))

    out = np.zeros((N, D), dtype=np.float32)
    res = None
    for p in range(n_pass):
        idx_p = [t[p * len(t) // n_pass : (p + 1) * len(t) // n_pass] for t in idx]
        wts_p = [w[p * len(w) // n_pass : (p + 1) * len(w) // n_pass] for w in wts]
        res = _run_pass(xf, W1, W2, idx_p, wts_p, out, trace)
    return out.reshape(B, T, D), res


def kernel(x, Wr, W1, W2):
    out, _ = _run(x, Wr, W1, W2, trace=False)
    return out
